# revision 1
# baseline (speedup 1.0000x reference)
"""GCN (4-layer, improved self-loops) on 8 Trainium2 NeuronCores.

Sharding: 1D node partition (6250 nodes/core); edges partitioned by
destination-node owner; per layer the prescaled features t_hat = dinv * (h@Wg)
are AllGathered into a full [50000, 128] DRAM table on every core, then each
core gathers per-edge source rows with dma_gather and scatter-adds them into
per-destination-block PSUM tiles via one-hot matmuls on the TensorEngine.

Self-loops (weight 2.0) are explicit edges, so the aggregation is one uniform
edge stream:
    h_next = elu(dinv[d] * sum_e w_e * t_hat[src_e] + b),  t_hat = dinv * t
which matches the reference exactly (norm_e = w_e * dinv[src] * dinv[dst] and
self_coef = 2 * dinv^2 both factor into the dinv sandwich).  deg/dinv are
O(E) scalar preprocessing computed on the host along with the edge partition.

On-chip node features are feature-major [H=128 partitions, nodes free]; the
aggregation matmul  PSUM[H, dst256] += V_tile^T @ Sw  (lhsT = gathered edge
rows, rhs = one-hot(dst_rel) * w built on the vector engine from iota +
metadata) lands feature-major again, so no transposes are needed between
layers.  Scatter/dense matmuls run as float32r (single-pass fp32) for 4x PE
row rate at moving dim >= 256.  Gather indices are int16, so the node table
is split at row 32768 into lo/hi streams.
"""

import numpy as np
from contextlib import ExitStack

try:
    import concourse.bass as bass
except ImportError:  # pragma: no cover
    import sys

    sys.path.insert(0, "/opt/trn_rl_repo")
    import concourse.bass as bass

import concourse.bacc as bacc
import concourse.mybir as mybir
import concourse.tile as tile
from concourse.bass_utils import run_bass_kernel_spmd

FP = mybir.dt.float32
FPR = mybir.dt.float32r
BF = mybir.dt.bfloat16
I16 = mybir.dt.int16

N = 50000
E = 800000
IN_D = 64
H = 128
OUT_D = 16
P = 8
NC_N = N // P            # 6250 nodes per core
BW = 256                 # destination-block width (scatter matmul moving dim)
NBLK = 25                # destination blocks per core (NPAD / BW)
NPAD = NBLK * BW         # 6400
NB_NODE = NPAD // 128    # 50 node-blocks of 128 for the t_hat path
SPLIT = 32768            # lo/hi src split so gather indices fit in int16
C_TILES = 32             # 128-edge tiles per dma_gather call
MC = 64                  # tiles per metadata DMA chunk

# dense-matmul column chunks over the padded node dim
CHUNKS = [(k * 512, 512) for k in range(12)] + [(6144, 256)]

ALU = mybir.AluOpType
ACT_F = mybir.ActivationFunctionType


def _prep_edges(edge_index, edge_weight):
    """Host preprocessing: partition edges by dst owner, add self loops,
    compute deg/dinv, split lo/hi by src, group by BW-dst block, pad each
    (core, block, stream) group to a common (max-over-cores) tile count.

    Returns (tlo, thi, per_core); per_core[c] has loidx/hiidx/meta/dinvrep/
    dinvT arrays.
    """
    src = np.asarray(edge_index[0], dtype=np.int64)
    dst = np.asarray(edge_index[1], dtype=np.int64)
    w = np.asarray(edge_weight, dtype=np.float32)

    core = dst // NC_N
    drel = dst % NC_N

    deg_full = np.zeros(N, dtype=np.float64)
    np.add.at(deg_full, dst, w.astype(np.float64))
    dinv_full = 1.0 / np.sqrt(deg_full + 2.0)

    self_drel = np.arange(NPAD, dtype=np.int64)
    self_w = np.full(NPAD, 2.0, dtype=np.float32)

    groups = [[[None, None, None] for _ in range(NBLK)] for _ in range(P)]
    for c in range(P):
        m = core == c
        s_all = np.concatenate(
            [src[m], np.minimum(self_drel, NC_N - 1) + c * NC_N]
        )
        d_all = np.concatenate([drel[m], self_drel])
        w_all = np.concatenate([w[m], self_w])
        blk = d_all // BW
        rel = (d_all % BW).astype(np.float32)
        is_local = (s_all >= c * NC_N) & (s_all < (c + 1) * NC_N)
        lo = s_all < SPLIT
        for b in range(NBLK):
            mb = blk == b
            for s, ms in (
                (0, mb & is_local),
                (1, mb & ~is_local & lo),
                (2, mb & ~is_local & ~lo),
            ):
                base = c * NC_N if s == 0 else (0 if s == 1 else SPLIT)
                idx = s_all[ms] - base
                groups[c][b][s] = (idx.astype(np.int16), rel[ms], w_all[ms])

    tcnt = np.zeros((3, NBLK), dtype=np.int64)
    for b in range(NBLK):
        for s in range(3):
            for c in range(P):
                tcnt[s, b] = max(
                    tcnt[s, b], -(-len(groups[c][b][s][0]) // 128)
                )
        tcnt[0, b] = max(tcnt[0, b], 1)

    TS = [int(tcnt[s].sum()) for s in range(3)]
    TT = sum(TS)

    per_core = []
    for c in range(P):
        idx_bufs = [np.zeros(TS[s] * 128, dtype=np.int16) for s in range(3)]
        # meta[e] = (dst_rel, w) in PE consumption order: per block, local
        # tiles then remote-lo then remote-hi; padding entries keep w=0.
        meta_rel = np.zeros(TT * 128, dtype=np.float32)
        meta_w = np.zeros(TT * 128, dtype=np.float32)
        offs = [0, 0, 0]
        om = 0
        for b in range(NBLK):
            for s in range(3):
                idx, rel, ww = groups[c][b][s]
                n = len(idx)
                cnt = int(tcnt[s, b])
                off = offs[s]
                idx_bufs[s][off * 128 : off * 128 + n] = idx
                meta_rel[om * 128 : om * 128 + n] = rel
                meta_w[om * 128 : om * 128 + n] = ww
                om += cnt
                offs[s] += cnt

        # wrapped int16 index layout: idx i lives at [i % 16, i // 16],
        # replicated 8x along partitions (one stripe per Q7 core)
        wraps = [
            np.ascontiguousarray(np.tile(ib.reshape(-1, 16).T, (8, 1)))
            for ib in idx_bufs
        ]
        # meta in partition-major tile layout: edge t*128+p -> [p, 2t + {0,1}]
        meta = np.empty((128, 2 * TT), dtype=np.float32)
        meta[:, 0::2] = meta_rel.reshape(TT, 128).T
        meta[:, 1::2] = meta_w.reshape(TT, 128).T

        dinv_c = np.zeros(NPAD, dtype=np.float32)
        dinv_c[:NC_N] = dinv_full[c * NC_N : (c + 1) * NC_N]
        dinvrep = np.ascontiguousarray(
            np.broadcast_to(dinv_c, (128, NPAD))
        ).astype(np.float32)
        dinvT = np.ascontiguousarray(dinv_c.reshape(NB_NODE, 128).T)

        per_core.append(
            {
                "lcidx": wraps[0],
                "loidx": wraps[1],
                "hiidx": wraps[2],
                "meta": meta,
                "dinvrep": dinvrep,
                "dinvT": dinvT,
            }
        )

    return tcnt, per_core


def _build_program(tcnt, single_core=False):
    # single_core=True swaps the AllGather for a local DMA copy and builds a
    # 1-device module, so the cost-model TimelineSim (single-core only) can
    # profile the kernel; numerics of remote nodes are wrong in that mode.
    TS = [int(tcnt[s].sum()) for s in range(3)]
    TT = sum(TS)
    nc = bacc.Bacc(
        "TRN2",
        target_bir_lowering=False,
        debug=False,
        enable_asserts=False,
        num_devices=1 if single_core else P,
    )

    # ---- I/O -------------------------------------------------------------
    xT_d = nc.dram_tensor("xT", [IN_D, NC_N], FP, kind="ExternalInput")
    lcidx_d = nc.dram_tensor("lcidx", [128, TS[0] * 8], I16, kind="ExternalInput")
    loidx_d = nc.dram_tensor("loidx", [128, TS[1] * 8], I16, kind="ExternalInput")
    hiidx_d = nc.dram_tensor("hiidx", [128, TS[2] * 8], I16, kind="ExternalInput")
    meta_d = nc.dram_tensor("meta", [128, 2 * TT], FP, kind="ExternalInput")
    dinvrep_d = nc.dram_tensor("dinvrep", [128, NPAD], FP, kind="ExternalInput")
    dinvT_d = nc.dram_tensor("dinvT", [128, NB_NODE], FP, kind="ExternalInput")
    w_d = {
        name: nc.dram_tensor(name, shape, FP, kind="ExternalInput")
        for name, shape in [
            ("W1", [IN_D, H]),
            ("W2", [H, H]),
            ("W3", [H, H]),
            ("Wg1", [H, H]),
            ("Wg2", [H, H]),
            ("Wg3", [H, H]),
            ("Wg4", [H, H]),
            ("Wh", [H, OUT_D]),
        ]
    }
    # bias columns: 0..2 = b1..b3, 3..6 = bg1..bg4, 7..13 = negated, 14 = bh
    bias_d = nc.dram_tensor("bias", [128, 16], FP, kind="ExternalInput")
    iota_d = nc.dram_tensor("iota256", [128, BW], BF, kind="ExternalInput")
    out_d = nc.dram_tensor("out", [OUT_D, NC_N], FP, kind="ExternalOutput")

    rg = [list(range(P))]

    with tile.TileContext(nc) as tc, ExitStack() as ctx:
        const = ctx.enter_context(tc.tile_pool(name="const", bufs=1))
        big = ctx.enter_context(tc.tile_pool(name="big", bufs=1))
        swp = ctx.enter_context(tc.tile_pool(name="swp", bufs=48))
        epp = ctx.enter_context(tc.tile_pool(name="epp", bufs=2))
        idxp = ctx.enter_context(tc.tile_pool(name="idxp", bufs=2))
        vlc_p = ctx.enter_context(tc.tile_pool(name="vlc", bufs=3))
        vlo_p = ctx.enter_context(tc.tile_pool(name="vlo", bufs=3))
        vhi_p = ctx.enter_context(tc.tile_pool(name="vhi", bufs=3))
        metap = ctx.enter_context(tc.tile_pool(name="metap", bufs=3))
        ps_dense = ctx.enter_context(tc.tile_pool(name="psd", bufs=2, space="PSUM"))
        ps_blk = ctx.enter_context(tc.tile_pool(name="psb", bufs=2, space="PSUM"))
        ps_tr = ctx.enter_context(tc.tile_pool(name="pst", bufs=2, space="PSUM"))
        dram = ctx.enter_context(tc.tile_pool(name="dram", bufs=2, space="DRAM"))

        # ---- constants ----------------------------------------------------
        def load_const(shape, src_ap, name, dtype=FP):
            t = const.tile(shape, dtype, tag=name)
            nc.sync.dma_start(t[:], src_ap)
            return t

        w_sb = {k: load_const(list(v.shape), v[:], k) for k, v in w_d.items()}
        bias = load_const([128, 16], bias_d[:], "bias")
        iota = load_const([128, BW], iota_d[:], "iota", BF)
        dinvT = load_const([128, NB_NODE], dinvT_d[:], "dinvT")

        h_sb = big.tile([128, NPAD], FP, tag="h")
        dinvrep = big.tile([128, NPAD], FP, tag="dinvrep")
        nc.sync.dma_start(dinvrep[:], dinvrep_d[:])

        # consumption-order bookkeeping
        s_of = [[], [], []]
        meta_of = []
        offs = [0, 0, 0]
        om = 0
        for b in range(NBLK):
            meta_of.append(om)
            for s in range(3):
                s_of[s].append(offs[s])
                offs[s] += int(tcnt[s, b])
                om += int(tcnt[s, b])

        n_mchunk = -(-TT // MC)

        def emit_meta_chunks():
            mts = []
            for i in range(n_mchunk):
                cols = min(MC, TT - i * MC)
                mt = metap.tile([128, 2 * MC], FP, tag="meta")
                nc.sync.dma_start(
                    mt[:, : 2 * cols], meta_d[:, 2 * i * MC : 2 * (i * MC + cols)]
                )
                mts.append(mt)
            return mts

        def sw_tile(mts, g):
            """[128 edge, BW dst] one-hot(dst_rel)*w scatter tile for
            consumption-order tile g, built on the vector engine."""
            mt = mts[g // MC]
            o = 2 * (g % MC)
            sw = swp.tile([128, BW], BF, tag="sw")
            nc.vector.tensor_scalar(
                sw[:],
                iota[:],
                mt[:, o : o + 1],
                mt[:, o + 1 : o + 2],
                ALU.is_equal,
                ALU.mult,
            )
            return sw

        # ---- embedding MLP -------------------------------------------------

        def elu_ep(dst_ap, ps_ap, bcol, cw):
            # DVE-heavy ELU: r = max(x+b, 0), m = min(x+b, 0) on DVE,
            # e = exp(m) on ACT, out = (e-1) + r on DVE.
            r = epp.tile([128, 512], FP, tag="r")
            nm = epp.tile([128, 512], FP, tag="nm")
            e2 = epp.tile([128, 512], FP, tag="e2")
            nc.vector.tensor_scalar(
                r[:, :cw], ps_ap, bias[:, bcol : bcol + 1], 0.0, ALU.add, ALU.max
            )
            nc.vector.tensor_scalar(
                nm[:, :cw], ps_ap, bias[:, bcol : bcol + 1], 0.0, ALU.add, ALU.min
            )
            nc.scalar.activation(e2[:, :cw], nm[:, :cw], ACT_F.Exp)
            nc.vector.scalar_tensor_tensor(
                dst_ap, e2[:, :cw], -1.0, r[:, :cw], ALU.add, ALU.add
            )

        for off, cw in CHUNKS:
            xc = epp.tile([IN_D, 512], FP, tag="xc")
            real = max(0, min(cw, NC_N - off))
            if real < cw:
                nc.vector.memset(xc[:, :cw], 0.0)
            if real > 0:
                nc.sync.dma_start(xc[:, :real], xT_d[:, off : off + real])
            ps = ps_dense.tile([128, 512], FP, tag="dense")
            nc.tensor.matmul(
                ps[:, :cw], w_sb["W1"][:IN_D, :], xc[:IN_D, :cw]
            )
            elu_ep(h_sb[:, off : off + cw], ps[:, :cw], 0, cw)
        for wname, bcol in [("W2", 1), ("W3", 2)]:
            for off, cw in CHUNKS:
                ps = ps_dense.tile([128, 512], FP, tag="dense")
                nc.tensor.matmul(
                    ps[:, :cw], w_sb[wname][:], h_sb[:, off : off + cw]
                )
                elu_ep(h_sb[:, off : off + cw], ps[:, :cw], bcol, cw)

        # ---- GCN layers ---------------------------------------------------
        n_chunk = [-(-TS[s] // C_TILES) for s in range(3)]

        for layer in range(4):
            wg = w_sb[f"Wg{layer + 1}"]
            bcol = 3 + layer

            # t-block (node-major) = h_blk^T @ Wg, prescale by dinv, write to
            # the AllGather input.  lhsT = h slice puts nodes on the output
            # partition axis directly, so no transposes are needed.
            agin = dram.tile([NC_N, H], BF, tag="agin")
            tfull = dram.tile([N, H], BF, tag="tfull", addr_space="Shared")
            for b in range(NB_NODE):
                rows = min(128, NC_N - b * 128)
                if rows <= 0:
                    continue
                trp = ps_tr.tile([128, 128], FP, tag="tr")
                nc.tensor.matmul(trp[:], h_sb[:, b * 128 : (b + 1) * 128], wg[:])
                tt = epp.tile([128, 128], BF, tag="tt")
                nc.scalar.activation(
                    tt[:], trp[:], ACT_F.Copy, scale=dinvT[:, b : b + 1]
                )
                nc.sync.dma_start(agin[b * 128 : b * 128 + rows, :], tt[:rows, :])

            if single_core:
                nc.sync.dma_start(tfull[:NC_N, :], agin[:])
            else:
                nc.gpsimd.collective_compute(
                    "AllGather",
                    ALU.bypass,
                    replica_groups=rg,
                    ins=[agin[:]],
                    outs=[tfull[:]],
                )

            def emit_gathers(nchunk, total_tiles, idx_dram, table_ap, pool, tag):
                chunks = []
                for i in range(nchunk):
                    nt = min(C_TILES, total_tiles - i * C_TILES)
                    it = idxp.tile([128, C_TILES * 8], I16, tag=f"i{tag}")
                    nc.sync.dma_start(
                        it[:, : nt * 8],
                        idx_dram[:, i * C_TILES * 8 : i * C_TILES * 8 + nt * 8],
                    )
                    v = pool.tile([128, C_TILES, 128], BF, tag=tag)
                    nc.gpsimd.dma_gather(
                        v[:, :nt, :], table_ap, it[:, : nt * 8],
                        nt * 128, nt * 128, H, single_packet=False,
                    )
                    chunks.append(v)
                return chunks

            vlc = emit_gathers(
                n_chunk[0], TS[0], lcidx_d, agin[:, :], vlc_p, "vlc"
            )
            vlo = emit_gathers(
                n_chunk[1], TS[1], loidx_d, tfull[:, :], vlo_p, "vlo"
            )
            vhi = emit_gathers(
                n_chunk[2], TS[2], hiidx_d, tfull[SPLIT:, :], vhi_p, "vhi"
            )
            vstreams = (vlc, vlo, vhi)
            mts = emit_meta_chunks()

            # per-block scatter-accumulate + epilogue (local tiles first:
            # they are ready before the AllGather completes)
            for b in range(NBLK):
                nt_s = [int(tcnt[s, b]) for s in range(3)]
                ntile = sum(nt_s)
                agg = ps_blk.tile([128, BW], FP, tag="agg")
                t = 0
                for s in range(3):
                    for k in range(nt_s[s]):
                        sw = sw_tile(mts, meta_of[b] + t)
                        g = s_of[s][b] + k
                        v = vstreams[s][g // C_TILES][:, g % C_TILES, :]
                        nc.tensor.matmul(
                            agg[:], v, sw[:],
                            start=(t == 0), stop=(t == ntile - 1),
                        )
                        t += 1
                vv = epp.tile([128, BW], FP, tag="vv")
                nc.vector.tensor_tensor(
                    vv[:], agg[:], dinvrep[:, b * BW : (b + 1) * BW], ALU.mult
                )
                rb = epp.tile([128, BW], FP, tag="rb")
                nmb = epp.tile([128, BW], FP, tag="nmb")
                eb = epp.tile([128, BW], FP, tag="eb")
                nc.scalar.activation(
                    rb[:], vv[:], ACT_F.Relu, bias=bias[:, bcol : bcol + 1]
                )
                nc.scalar.activation(
                    nmb[:], vv[:], ACT_F.Relu,
                    bias=bias[:, bcol + 7 : bcol + 8], scale=-1.0,
                )
                nc.scalar.activation(eb[:], nmb[:], ACT_F.Exp, scale=-1.0)
                nc.vector.scalar_tensor_tensor(
                    h_sb[:, b * BW : (b + 1) * BW],
                    eb[:], -1.0, rb[:], ALU.add, ALU.add,
                )

        # ---- head ----------------------------------------------------------
        for off, cw in CHUNKS:
            cw = min(cw, NC_N - off)
            ps = ps_dense.tile([128, 512], FP, tag="dense")
            nc.tensor.matmul(
                ps[:OUT_D, :cw], w_sb["Wh"][:], h_sb[:, off : off + cw]
            )
            oc = epp.tile([OUT_D, 512], FP, tag="outc")
            nc.scalar.activation(
                oc[:, :cw], ps[:OUT_D, :cw], ACT_F.Identity,
                bias=bias[:OUT_D, 14:15],
            )
            nc.sync.dma_start(out_d[:, off : off + cw], oc[:, :cw])

    nc.compile()
    return nc


def _make_in_maps(inputs, per_core):
    x = np.asarray(inputs["x"], dtype=np.float32)
    bias = np.zeros((128, 16), dtype=np.float32)
    for j, nm in enumerate(["b1", "b2", "b3", "bg1", "bg2", "bg3", "bg4"]):
        b = np.asarray(inputs[nm], dtype=np.float32)
        bias[:, j] = b
        bias[:, j + 7] = -b
    bias[:OUT_D, 14] = np.asarray(inputs["bh"], dtype=np.float32)

    import ml_dtypes

    shared = {
        "bias": bias,
        "iota256": np.tile(
            np.arange(BW, dtype=np.float32), (128, 1)
        ).astype(ml_dtypes.bfloat16),
    }
    for nm in ["W1", "W2", "W3", "Wg1", "Wg2", "Wg3", "Wg4", "Wh"]:
        shared[nm] = np.ascontiguousarray(np.asarray(inputs[nm], np.float32))

    in_maps = []
    for c in range(P):
        m = dict(shared)
        m["xT"] = np.ascontiguousarray(x[c * NC_N : (c + 1) * NC_N].T)
        m.update(per_core[c])
        in_maps.append(m)
    return in_maps


def run(inputs, trace=False):
    """Run the distributed kernel; returns (out [N, OUT_D] fp32, results)."""
    tcnt, per_core = _prep_edges(inputs["edge_index"], inputs["edge_weight"])
    nc = _build_program(tcnt)
    in_maps = _make_in_maps(inputs, per_core)
    res = run_bass_kernel_spmd(nc, in_maps, list(range(P)), trace=trace)
    out = np.concatenate(
        [res.results[c]["out"].T for c in range(P)], axis=0
    ).astype(np.float32)
    return out, res


def kernel(**inputs):
    out, _ = run(inputs, trace=False)
    return out



# revision 30
# speedup vs baseline: 1.5807x; 1.5807x over previous
"""GCN (4-layer, improved self-loops) on 8 Trainium2 NeuronCores.

Sharding: 1D node partition (6250 nodes/core); edges partitioned by
destination-node owner; per layer the prescaled features t_hat = dinv * (h@Wg)
are AllGathered into a full [50000, 128] DRAM table on every core, then each
core gathers per-edge source rows with dma_gather and scatter-adds them into
per-destination-block PSUM tiles via one-hot matmuls on the TensorEngine.

Self-loops (weight 2.0) are explicit edges and dinv[dst] is folded into the
per-edge weight on the host (w' = w * dinv[dst], self w' = 2 * dinv[d]), so
the aggregation is one uniform weighted edge stream over t_hat = dinv * t and
the PSUM result is the final pre-bias value:
    h_next = elu(sum_e w'_e * t_hat[src_e] + b)

Edges are packed densely per (dst-block, stream) group with group lengths
uniform across cores (max over cores, padded with w'=0 slots); groups start at
arbitrary 128-alignment offsets, so boundary matmuls use partition subranges
of the gathered tiles.  Streams: local (incl. self loops, gathered from the
core's own agin table), remote-lo (src < 32768) and remote-hi, since gather
indices are signed int16.

On-chip node features are feature-major bf16 [H=128 partitions, nodes free];
the aggregation matmul  PSUM[H, 128 dst] += V^T @ Sw  uses lhsT = gathered
edge rows and rhs = one-hot(dst_rel) * w' built on the vector engine from
iota + resident metadata.  idx/meta are layer-invariant and live in SBUF for
the whole kernel.  The next layer's t = h@Wg phase is interleaved into the
scatter loop (2-block lag) so table building overlaps the gather tail.
"""

import numpy as np
from contextlib import ExitStack

try:
    import concourse.bass as bass
except ImportError:  # pragma: no cover
    import sys

    sys.path.insert(0, "/opt/trn_rl_repo")
    import concourse.bass as bass

import concourse.bacc as bacc
import concourse.mybir as mybir
import concourse.tile as tile
from concourse.bass_utils import run_bass_kernel_spmd

FP = mybir.dt.float32
BF = mybir.dt.bfloat16
I16 = mybir.dt.int16

N = 50000
E = 800000
IN_D = 64
H = 128
OUT_D = 16
P = 8
NC_N = N // P            # 6250 nodes per core
BW = 128                 # destination-block width (scatter matmul moving dim)
NBLK = 49                # destination blocks per core
NPAD = NBLK * BW         # 6272
HI_BASE = N - 32768      # hi-stream table base; lo covers [0, 32768) and hi
                         # covers [HI_BASE, N) so signed-int16 gather indices
                         # reach every row, and the overlap [HI_BASE, 32768)
                         # is assigned per-core to balance lo/hi stream sizes
                         # (max-over-cores group padding then is noise only)
C_TILES = 32             # 128-edge tiles per dma_gather call
TLAG = 2                 # scatter->next-t-phase interleave lag (blocks)

# dense-matmul column chunks over the node dim (MLP / head / t-phase groups)
CHUNKS = [(k * 512, min(512, NC_N - k * 512)) for k in range(13)]

ALU = mybir.AluOpType
ACT_F = mybir.ActivationFunctionType


def _prep_edges(edge_index, edge_weight):
    """Host preprocessing: partition edges by dst owner, fold dinv into
    per-edge weights, add self loops, split streams (local / remote-lo /
    remote-hi), group by 128-wide dst block with group sizes uniform across
    cores, and pack densely (no per-group tile padding).

    Returns (plan, per_core).
    """
    src = np.asarray(edge_index[0], dtype=np.int64)
    dst = np.asarray(edge_index[1], dtype=np.int64)
    w = np.asarray(edge_weight, dtype=np.float64)

    core = dst // NC_N
    drel = dst % NC_N

    deg_full = np.zeros(N, dtype=np.float64)
    np.add.at(deg_full, dst, w)
    dinv_full = 1.0 / np.sqrt(deg_full + 2.0)

    wprime = w * dinv_full[dst]                      # fold dinv[dst] in

    # self-loops are NOT edges here: they are applied as one extra matmul
    # per dst block against the locally-kept node-major t_hat (diag tiles)

    # groups[c][b][s] = (idx int64 rel-to-stream-base, rel f32, w f32)
    groups = [[[None] * 3 for _ in range(NBLK)] for _ in range(P)]
    for c in range(P):
        m = core == c
        s_all = src[m]
        d_all = drel[m]
        w_all = wprime[m].astype(np.float32)
        blk = d_all // BW
        rel = (d_all % BW).astype(np.float32)
        is_local = (s_all >= c * NC_N) & (s_all < (c + 1) * NC_N)
        for b in range(NBLK):
            mb = blk == b
            mloc = mb & is_local
            mrem = mb & ~is_local
            ridx = np.nonzero(mrem)[0]
            rsrc = s_all[ridx]
            # balance lo/hi: overlap rows [HI_BASE, 32768) go to whichever
            # stream is short on this (core, block)
            fixed_lo = rsrc < HI_BASE
            fixed_hi = rsrc >= 32768
            mid = ~fixed_lo & ~fixed_hi
            n_lo = int(np.clip(len(rsrc) // 2, fixed_lo.sum(),
                               fixed_lo.sum() + mid.sum()))
            take = n_lo - int(fixed_lo.sum())
            mid_idx = np.nonzero(mid)[0]
            to_lo = fixed_lo.copy()
            to_lo[mid_idx[:take]] = True
            for s, ms in (
                (0, np.nonzero(mloc)[0]),
                (1, ridx[to_lo]),
                (2, ridx[~to_lo]),
            ):
                base = c * NC_N if s == 0 else (0 if s == 1 else HI_BASE)
                order = np.argsort(s_all[ms], kind="stable")
                groups[c][b][s] = (
                    (s_all[ms] - base)[order],
                    rel[ms][order],
                    w_all[ms][order],
                )

    # uniform group lengths = max over cores
    n_g = np.zeros((3, NBLK), dtype=np.int64)
    for b in range(NBLK):
        for s in range(3):
            n_g[s, b] = max(len(groups[c][b][s][0]) for c in range(P))

    # group offsets within each packed stream; matmul schedule per block.
    # Boundary tiles shared by two blocks are consumed once per block with
    # full 128 partitions; the other block's edges carry w'=0 in that op's
    # meta column (PE cost depends only on the moving dim, so this is free).
    offs = [0, 0, 0]
    o_g = np.zeros((3, NBLK), dtype=np.int64)
    sched = []          # sched[b] = [(s, tile_col, op_index), ...]
    nop = 0
    for b in range(NBLK):
        ops = []
        for s in range(3):
            o, n = offs[s], int(n_g[s, b])
            o_g[s, b] = o
            offs[s] += n
            if n == 0:
                continue
            for tc_ in range(o // 128, (o + n - 1) // 128 + 1):
                ops.append((s, tc_, nop))
                nop += 1
        sched.append(ops)

    T_s = [int(offs[s]) for s in range(3)]            # packed edge slots
    TS = [-(-T_s[s] // 128) for s in range(3)]        # stream tiles

    # gather chunk lists + issue order (lc first, then lo/hi merged by the
    # first block that consumes each chunk)
    def first_block(s, tile0):
        pos = tile0 * 128
        for b in range(NBLK):
            if pos < o_g[s, b] + n_g[s, b]:
                return b
        return NBLK

    chunk_list = []                                   # (s, tile0, ntiles)
    remote = []
    for s in range(3):
        for t0 in range(0, TS[s], C_TILES):
            nt = min(C_TILES, TS[s] - t0)
            if s == 0:
                chunk_list.append((s, t0, nt))
            else:
                remote.append((first_block(s, t0), s, t0, nt))
    remote.sort()
    chunk_list += [(s, t0, nt) for _, s, t0, nt in remote]

    plan = {"n_g": n_g, "o_g": o_g, "sched": sched, "T_s": T_s, "TS": TS,
            "chunks": chunk_list, "nop": nop}

    per_core = []
    for c in range(P):
        idxs = []
        srel = []                      # per-stream per-slot rel / w'
        sww = []
        for s in range(3):
            nbuf = TS[s] * 128
            ib = np.zeros(nbuf, dtype=np.int16)
            mrel = np.zeros(nbuf, dtype=np.float32)
            mw = np.zeros(nbuf, dtype=np.float32)
            for b in range(NBLK):
                idx, rel, ww = groups[c][b][s]
                o, n = int(o_g[s, b]), len(idx)
                ib[o : o + n] = idx.astype(np.int16)
                mrel[o : o + n] = rel
                mw[o : o + n] = ww
            # wrapped int16 index layout: idx i at [i % 16, i // 16],
            # replicated 8x along partitions (one stripe per Q7 core)
            idxs.append(
                np.ascontiguousarray(np.tile(ib.reshape(-1, 16).T, (8, 1)))
            )
            srel.append(mrel)
            sww.append(mw)

        # meta per consumption-order op: [p, 2i] = rel, [p, 2i+1] = w', with
        # w'=0 for slots outside the op's (block, stream) group
        meta = np.zeros((128, 2 * nop), dtype=np.float32)
        for b in range(NBLK):
            for s, tc_, i in sched[b]:
                slots = np.arange(tc_ * 128, tc_ * 128 + 128)
                inside = (slots >= o_g[s, b]) & (slots < o_g[s, b] + n_g[s, b])
                meta[:, 2 * i] = np.where(inside, srel[s][slots], 0.0)
                meta[:, 2 * i + 1] = np.where(inside, sww[s][slots], 0.0)

        dinv_c = np.zeros(NPAD, dtype=np.float32)
        dinv_c[:NC_N] = dinv_full[c * NC_N : (c + 1) * NC_N]
        dinvT = np.ascontiguousarray(dinv_c.reshape(NBLK, 128).T)

        per_core.append(
            {
                "lcidx": idxs[0], "loidx": idxs[1], "hiidx": idxs[2],
                "meta": meta, "dinvT": dinvT,
            }
        )

    return plan, per_core


def _build_program(plan, single_core=False):
    # single_core=True swaps the AllGather for a local DMA copy and builds a
    # 1-device module, so the cost-model TimelineSim (single-core only) can
    # profile the kernel; numerics of remote nodes are wrong in that mode.
    TS = plan["TS"]
    sched = plan["sched"]
    chunk_list = plan["chunks"]
    nc = bacc.Bacc(
        "TRN2",
        target_bir_lowering=False,
        debug=False,
        enable_asserts=False,
        num_devices=1 if single_core else P,
    )

    # ---- I/O -------------------------------------------------------------
    xT_d = nc.dram_tensor("xT", [IN_D, NC_N], BF, kind="ExternalInput")
    idx_d = {
        nm: nc.dram_tensor(nm, [128, TS[s] * 8], I16, kind="ExternalInput")
        for s, nm in enumerate(["lcidx", "loidx", "hiidx"])
    }
    nop = plan["nop"]
    meta_d = nc.dram_tensor("meta", [128, 2 * nop], FP, kind="ExternalInput")
    dinvT_d = nc.dram_tensor("dinvT", [128, NBLK], FP, kind="ExternalInput")
    w_d = {
        name: nc.dram_tensor(name, shape, BF, kind="ExternalInput")
        for name, shape in [
            ("W1", [IN_D, H]),
            ("W2", [H, H]),
            ("W3", [H, H]),
            ("Wg1", [H, H]),
            ("Wg2", [H, H]),
            ("Wg3", [H, H]),
            ("Wg4", [H, H]),
            ("Wh", [H, OUT_D]),
        ]
    }
    # bias columns: 0..2 = b1..b3, 3..6 = bg1..bg4, 7..13 = negated, 14 = bh
    bias_d = nc.dram_tensor("bias", [128, 16], FP, kind="ExternalInput")
    iota_d = nc.dram_tensor("iota128", [128, BW], BF, kind="ExternalInput")
    ident2_d = nc.dram_tensor("ident2", [128, 128], BF, kind="ExternalInput")
    out_d = nc.dram_tensor("out", [OUT_D, NC_N], FP, kind="ExternalOutput")

    rg = [list(range(P))]

    with tile.TileContext(nc) as tc, ExitStack() as ctx:
        const = ctx.enter_context(tc.tile_pool(name="const", bufs=1))
        big = ctx.enter_context(tc.tile_pool(name="big", bufs=1))
        swp = ctx.enter_context(tc.tile_pool(name="swp", bufs=16))
        epp = ctx.enter_context(tc.tile_pool(name="epp", bufs=3))
        ebp = ctx.enter_context(tc.tile_pool(name="ebp", bufs=6))
        vlc_p = ctx.enter_context(tc.tile_pool(name="vlc", bufs=3))
        vlo_p = ctx.enter_context(tc.tile_pool(name="vlo", bufs=3))
        vhi_p = ctx.enter_context(tc.tile_pool(name="vhi", bufs=3))
        ps_dense = ctx.enter_context(tc.tile_pool(name="psd", bufs=2, space="PSUM"))
        ps_blk = ctx.enter_context(tc.tile_pool(name="psb", bufs=3, space="PSUM"))
        ps_tr = ctx.enter_context(tc.tile_pool(name="pst", bufs=2, space="PSUM"))
        dram = ctx.enter_context(tc.tile_pool(name="dram", bufs=2, space="DRAM"))

        # ---- constants (idx/meta are layer-invariant: resident in SBUF) ---
        def load_const(shape, src_ap, name, dtype=FP):
            t = const.tile(shape, dtype, tag=name)
            nc.sync.dma_start(t[:], src_ap)
            return t

        w_sb = {k: load_const(list(v.shape), v[:], k, BF) for k, v in w_d.items()}
        bias = load_const([128, 16], bias_d[:], "bias")
        iota = load_const([128, BW], iota_d[:], "iota", BF)
        dinvT = load_const([128, NBLK], dinvT_d[:], "dinvT")
        idx_sb = [
            load_const([128, TS[s] * 8], idx_d[nm][:], nm, I16)
            for s, nm in enumerate(["lcidx", "loidx", "hiidx"])
        ]
        meta_sb = load_const([128, 2 * nop], meta_d[:], "meta")
        ident2 = load_const([128, 128], ident2_d[:], "ident2", BF)

        h_sb = big.tile([128, NPAD], BF, tag="h")
        # two node-major t_hat buffers (layer parity): feed the self-loop
        # matmuls and the agin DMA without a DRAM round trip
        t_loc = [
            big.tile([128, NPAD], BF, tag=f"tloc{i}", name=f"tloc{i}")
            for i in range(2)
        ]
        # diag_b = 2*dinv*I, the self-loop scatter tiles (layer-invariant)
        diag_sb = []
        for b in range(NBLK):
            d = const.tile([128, 128], BF, tag=f"diag{b}", name=f"diag{b}")
            nc.vector.tensor_scalar(
                d[:], ident2[:], dinvT[:, b : b + 1], 0.0, ALU.mult, ALU.add
            )
            diag_sb.append(d)

        def sw_build(sw4, j, i):
            """Build the one-hot(dst_rel)*w' scatter tile for consumption-
            order op i into slice j of a shared [128, 512] tile (one pool
            ring slot per 4 builds keeps the DVE sequencer clear)."""
            o = 2 * i
            nc.vector.tensor_scalar(
                sw4[:, j * BW : (j + 1) * BW],
                iota[:],
                meta_sb[:, o : o + 1],
                meta_sb[:, o + 1 : o + 2],
                ALU.is_equal,
                ALU.mult,
            )

        # ---- embedding MLP -------------------------------------------------

        def elu_ep(dst_ap, ps_ap, bcol, cw, mode):
            # ELU: out = relu(x+b) + (exp(min(x+b, 0)) - 1).  Work rotates
            # over DVE / ACT / Pool per chunk so no single engine binds.
            r = epp.tile([128, 512], BF, tag="r")
            e2 = epp.tile([128, 512], BF, tag="e2")
            if mode == "act":
                nm = epp.tile([128, 512], BF, tag="nm")
                nc.scalar.activation(
                    r[:, :cw], ps_ap, ACT_F.Relu, bias=bias[:, bcol : bcol + 1]
                )
                nc.scalar.activation(
                    nm[:, :cw], ps_ap, ACT_F.Relu,
                    bias=bias[:, bcol + 7 : bcol + 8], scale=-1.0,
                )
                nc.scalar.activation(e2[:, :cw], nm[:, :cw], ACT_F.Exp, scale=-1.0)
                nc.vector.scalar_tensor_tensor(
                    dst_ap, e2[:, :cw], -1.0, r[:, :cw], ALU.add, ALU.add
                )
                return
            if mode == "pool":
                # gpsimd cannot read PSUM: ACT makes a biased SBUF copy first
                tb = epp.tile([128, 512], BF, tag="tb")
                nm = epp.tile([128, 512], BF, tag="nm")
                nc.scalar.activation(
                    tb[:, :cw], ps_ap, ACT_F.Identity,
                    bias=bias[:, bcol : bcol + 1],
                )
                nc.gpsimd.tensor_scalar(
                    r[:, :cw], tb[:, :cw], 0.0, 0.0, ALU.max, ALU.add
                )
                nc.gpsimd.tensor_scalar(
                    nm[:, :cw], tb[:, :cw], 0.0, 0.0, ALU.min, ALU.add
                )
                nc.scalar.activation(e2[:, :cw], nm[:, :cw], ACT_F.Exp)
                nc.gpsimd.scalar_tensor_tensor(
                    dst_ap, e2[:, :cw], -1.0, r[:, :cw], ALU.add, ALU.add
                )
                return
            nm = epp.tile([128, 512], FP, tag="nmf")
            nc.vector.tensor_scalar(
                r[:, :cw], ps_ap, bias[:, bcol : bcol + 1], 0.0,
                ALU.add, ALU.max,
            )
            nc.vector.tensor_scalar(
                nm[:, :cw], ps_ap, bias[:, bcol : bcol + 1], 0.0,
                ALU.add, ALU.min,
            )
            nc.scalar.activation(e2[:, :cw], nm[:, :cw], ACT_F.Exp)
            nc.vector.scalar_tensor_tensor(
                dst_ap, e2[:, :cw], -1.0, r[:, :cw], ALU.add, ALU.add
            )

        # ---- t-phase (t_hat table build), interleavable in 4-block groups -
        def emit_t_group(g4, wg, agin, tl):
            b0 = 4 * g4
            blocks = range(b0, min(b0 + 4, NBLK))
            for nb in blocks:
                trp = ps_tr.tile([128, 128], FP, tag="tr")
                nc.tensor.matmul(
                    trp[:], h_sb[:, nb * 128 : (nb + 1) * 128], wg[:]
                )
                nc.scalar.activation(
                    tl[:, nb * 128 : (nb + 1) * 128], trp[:], ACT_F.Copy,
                    scale=dinvT[:, nb : nb + 1],
                )
            rows = min(512, NC_N - b0 * 128)
            # node-major DRAM rows from partition-major SBUF: view the DRAM
            # side as [p, j, f] so dims match the SBUF tile layout
            if rows % 128 == 0:
                jn = rows // 128
                nc.sync.dma_start(
                    agin[b0 * 128 : b0 * 128 + rows, :].rearrange(
                        "(j p) f -> p j f", p=128
                    ),
                    tl[:, b0 * 128 : b0 * 128 + jn * 128].rearrange(
                        "p (j f) -> p j f", f=128
                    ),
                )
            else:
                nc.sync.dma_start(
                    agin[b0 * 128 : b0 * 128 + rows, :],
                    tl[:rows, b0 * 128 : (b0 + 1) * 128],
                )

        def emit_head_chunk(ch):
            off, cw = CHUNKS[ch]
            ps = ps_dense.tile([128, 512], FP, tag="dense")
            nc.tensor.matmul(
                ps[:OUT_D, :cw], w_sb["Wh"][:], h_sb[:, off : off + cw]
            )
            oc = epp.tile([OUT_D, 512], FP, tag="outc")
            nc.scalar.activation(
                oc[:, :cw], ps[:OUT_D, :cw], ACT_F.Identity,
                bias=bias[:OUT_D, 14:15],
            )
            nc.sync.dma_start(out_d[:, off : off + cw], oc[:, :cw])

        # ---- MLP with layer-0 t-phase interleaved -------------------------
        agin = [None] * 4
        tfull = [None] * 4
        agin[0] = dram.tile([NC_N, H], BF, tag="agin", name="agin0")

        for li, (wname, bcol) in enumerate([("W1", 0), ("W2", 1), ("W3", 2)]):
            for ci, (off, cw) in enumerate(CHUNKS):
                if li == 0:
                    xc = epp.tile([IN_D, 512], BF, tag="xc")
                    nc.sync.dma_start(xc[:, :cw], xT_d[:, off : off + cw])
                    rhs = xc[:IN_D, :cw]
                    lhs = w_sb["W1"][:IN_D, :]
                else:
                    rhs = h_sb[:, off : off + cw]
                    lhs = w_sb[wname][:]
                ps = ps_dense.tile([128, 512], FP, tag="dense")
                nc.tensor.matmul(ps[:, :cw], lhs, rhs)
                if cw < 512 and li == 2:
                    # zero pad columns of h for the scatter phase
                    nc.vector.memset(h_sb[:, off + cw : NPAD], 0.0)
                elu_ep(
                    h_sb[:, off : off + cw], ps[:, :cw], bcol, cw,
                    mode=("dve", "act")[ci % 2],
                )
                if li == 2 and ci >= 1:
                    emit_t_group(ci - 1, w_sb["Wg1"], agin[0], t_loc[0])
        emit_t_group(12, w_sb["Wg1"], agin[0], t_loc[0])

        # ---- GCN layers ---------------------------------------------------
        for layer in range(4):
            bcol = 3 + layer

            tfull[layer] = dram.tile([N, H], BF, tag="tfull", addr_space="Shared", name=f"tfull{layer}")
            if single_core:
                nc.sync.dma_start(tfull[layer][:NC_N, :], agin[layer][:])
            else:
                nc.gpsimd.collective_compute(
                    "AllGather",
                    ALU.bypass,
                    replica_groups=rg,
                    ins=[agin[layer][:]],
                    outs=[tfull[layer][:]],
                )

            # gathers in consumption-ish order (lc first: they only need agin)
            tabs = [agin[layer][:, :], tfull[layer][:, :], tfull[layer][HI_BASE:, :]]
            pools = [vlc_p, vlo_p, vhi_p]
            vchunks = [{}, {}, {}]          # tile0 -> (tile, nt)
            for s, t0, nt in chunk_list:
                v = pools[s].tile([128, C_TILES, 128], BF, tag=f"v{s}")
                nc.gpsimd.dma_gather(
                    v[:, :nt, :], tabs[s], idx_sb[s][:, t0 * 8 : (t0 + nt) * 8],
                    nt * 128, nt * 128, H, single_packet=False,
                )
                vchunks[s][t0] = v

            if layer < 3:
                agin[layer + 1] = dram.tile([NC_N, H], BF, tag="agin", name=f"agin{layer+1}")
                nwg = w_sb[f"Wg{layer + 2}"]

            # scatter + epilogue (deferred 1 block) + interleaved next t-phase
            aggs = [None] * NBLK

            def emit_epilogue(b):
                agg = aggs[b]
                rb = ebp.tile([128, BW], BF, tag="rb")
                nmb = ebp.tile([128, BW], BF, tag="nmb")
                eb = ebp.tile([128, BW], BF, tag="eb")
                nc.scalar.activation(
                    rb[:], agg[:], ACT_F.Relu, bias=bias[:, bcol : bcol + 1]
                )
                nc.scalar.activation(
                    nmb[:], agg[:], ACT_F.Relu,
                    bias=bias[:, bcol + 7 : bcol + 8], scale=-1.0,
                )
                nc.scalar.activation(eb[:], nmb[:], ACT_F.Exp, scale=-1.0)
                nc.vector.scalar_tensor_tensor(
                    h_sb[:, b * BW : (b + 1) * BW],
                    eb[:], -1.0, rb[:], ALU.add, ALU.add,
                )

            tl_cur = t_loc[layer % 2]
            for b in range(NBLK):
                ops = sched[b]
                agg = ps_blk.tile([128, BW], FP, tag="agg")
                aggs[b] = agg
                nops_b = len(ops)
                # self-loop term: agg = t_loc[b]^T @ (2*dinv*I)
                nc.tensor.matmul(
                    agg[:], tl_cur[:, b * 128 : (b + 1) * 128], diag_sb[b][:],
                    start=True, stop=(nops_b == 0),
                )
                sw4 = None
                for k, (s, tc_, i) in enumerate(ops):
                    j = k % 4
                    if j == 0:
                        sw4 = swp.tile([128, 4 * BW], BF, tag="sw")
                    sw_build(sw4, j, i)
                    t0 = (tc_ // C_TILES) * C_TILES
                    v = vchunks[s][t0]
                    nc.tensor.matmul(
                        agg[:], v[:, tc_ - t0, :],
                        sw4[:, j * BW : (j + 1) * BW],
                        start=False, stop=(k == nops_b - 1),
                    )
                if b >= 1:
                    emit_epilogue(b - 1)
                # interleaved t-phase of the next layer (or head chunks)
                if b >= TLAG and (b - TLAG + 1) % 4 == 0:
                    g4 = (b - TLAG + 1) // 4 - 1
                    if layer < 3:
                        emit_t_group(g4, nwg, agin[layer + 1],
                                     t_loc[(layer + 1) % 2])
                    else:
                        emit_head_chunk(g4)
            emit_epilogue(NBLK - 1)
            # trailing t-groups (blocks whose h finished at the very end)
            gfirst = (NBLK - TLAG) // 4  # first group not yet emitted
            for g4 in range(gfirst, 13):
                if layer < 3:
                    emit_t_group(g4, nwg, agin[layer + 1],
                                 t_loc[(layer + 1) % 2])
                else:
                    emit_head_chunk(g4)

    nc.compile()
    return nc


def _make_in_maps(inputs, per_core):
    import ml_dtypes

    x = np.asarray(inputs["x"], dtype=np.float32)
    bias = np.zeros((128, 16), dtype=np.float32)
    for j, nm in enumerate(["b1", "b2", "b3", "bg1", "bg2", "bg3", "bg4"]):
        b = np.asarray(inputs[nm], dtype=np.float32)
        bias[:, j] = b
        bias[:, j + 7] = -b
    bias[:OUT_D, 14] = np.asarray(inputs["bh"], dtype=np.float32)

    shared = {
        "bias": bias,
        "iota128": np.tile(
            np.arange(BW, dtype=np.float32), (128, 1)
        ).astype(ml_dtypes.bfloat16),
        "ident2": (2.0 * np.eye(128, dtype=np.float32)).astype(
            ml_dtypes.bfloat16
        ),
    }
    for nm in ["W1", "W2", "W3", "Wg1", "Wg2", "Wg3", "Wg4", "Wh"]:
        shared[nm] = np.ascontiguousarray(
            np.asarray(inputs[nm], np.float32)
        ).astype(ml_dtypes.bfloat16)

    in_maps = []
    for c in range(P):
        m = dict(shared)
        m["xT"] = np.ascontiguousarray(
            x[c * NC_N : (c + 1) * NC_N].T
        ).astype(ml_dtypes.bfloat16)
        m.update(per_core[c])
        in_maps.append(m)
    return in_maps


def run(inputs, trace=False):
    """Run the distributed kernel; returns (out [N, OUT_D] fp32, results)."""
    plan, per_core = _prep_edges(inputs["edge_index"], inputs["edge_weight"])
    nc = _build_program(plan)
    in_maps = _make_in_maps(inputs, per_core)
    res = run_bass_kernel_spmd(nc, in_maps, list(range(P)), trace=trace)
    out = np.concatenate(
        [res.results[c]["out"].T for c in range(P)], axis=0
    ).astype(np.float32)
    return out, res


def kernel(**inputs):
    out, _ = run(inputs, trace=False)
    return out


# revision 43
# speedup vs baseline: 1.6974x; 1.0738x over previous
"""GCN (4-layer, improved self-loops) on 8 Trainium2 NeuronCores.

Sharding: 1D node partition (6250 nodes/core); edges partitioned by
destination-node owner.  Per layer the prescaled features t_hat = dinv*(h@Wg)
are computed on-core (node-major bf16, kept in SBUF as t_loc), written to a
DRAM table with ONE contiguous partition-major DMA per 4-block group, and
AllGathered into a full table on every core; each core then gathers per-edge
source rows with dma_gather and scatter-adds them into per-dst-block PSUM
tiles via one-hot matmuls on the TensorEngine:
    PSUM[H, 128 dst] += V_edges^T @ Sw,   Sw = one-hot(dst_rel) * w'
with w' = w * dinv[dst] folded on the host, so PSUM holds the final pre-bias
value and h_next = elu(PSUM + b) directly.  Self-loops are NOT edges: the
2*dinv[d]^2*t[d] term is one extra matmul per block, t_loc^T @ (2*dinv*I),
against prebuilt diagonal tiles — no gather traffic.

The gather table rows are PERMUTED (node n of core c at row c*6272 +
(n%128)*49 + n//128 = the raw t_loc bytes), which makes the table-write DMAs
contiguous.  Edge streams: local (from the core's own agin), remote-lo
(rows < 32768) and remote-hi (rows >= N_TAB-32768), because gather indices
are signed int16; rows in the overlap are assigned per-core per-block to
BALANCE lo/hi sizes, so the max-over-cores uniform group padding is noise
only (~4%).  Within each stream, edges are packed densely; a 128-edge tile
spanning two dst blocks is consumed once per block with the foreign edges
zeroed via w'=0 in that op's meta column (PE cost depends only on the moving
dim).  idx/meta are layer-invariant and resident in SBUF.

The next layer's t-phase is interleaved into the scatter loop (TLAG-block
lag) so table building and agin writes overlap the gather tail; epilogues
are deferred one block to keep the DVE queue free for sw builds, and sw
tiles are allocated in groups of 4 to amortize pool-ring semaphores.  The
embedding MLP rotates its ELU between a bf16 fast path on DVE and ACT.
"""

import numpy as np
from contextlib import ExitStack

try:
    import concourse.bass as bass
except ImportError:  # pragma: no cover
    import sys

    sys.path.insert(0, "/opt/trn_rl_repo")
    import concourse.bass as bass

import concourse.bacc as bacc
import concourse.mybir as mybir
import concourse.tile as tile
from concourse.bass_utils import run_bass_kernel_spmd

FP = mybir.dt.float32
BF = mybir.dt.bfloat16
I16 = mybir.dt.int16

N = 50000
E = 800000
IN_D = 64
H = 128
OUT_D = 16
P = 8
NC_N = N // P            # 6250 nodes per core
BW = 128                 # destination-block width (scatter matmul moving dim)
NBLK = 49                # destination blocks per core
NPAD = NBLK * BW         # 6272
N_TAB = 8 * 6272         # rows in the AllGathered t_hat table (incl. pad)
HI_BASE = N_TAB - 32768  # hi-stream table base; lo covers [0, 32768) and hi
                         # covers [HI_BASE, N_TAB) so signed-int16 gather
                         # indices reach every row; the overlap [HI_BASE,
                         # 32768) is assigned per-core to balance lo/hi
                         # stream sizes (group padding becomes noise only).
                         # Table rows are PERMUTED: node n of core c sits at
                         # row c*6272 + (n%128)*49 + n//128 — the raw bytes of
                         # the partition-major t_loc SBUF tile, so agin writes
                         # are contiguous 1KB-per-partition DMAs.
C_TILES = 32             # 128-edge tiles per dma_gather call
TLAG = 2                 # scatter->next-t-phase interleave lag (blocks)

# dense-matmul column chunks over the node dim (MLP / head / t-phase groups)
CHUNKS = [(k * 512, min(512, NC_N - k * 512)) for k in range(13)]

ALU = mybir.AluOpType
ACT_F = mybir.ActivationFunctionType


def _prep_edges(edge_index, edge_weight):
    """Host preprocessing: partition edges by dst owner, fold dinv into
    per-edge weights, add self loops, split streams (local / remote-lo /
    remote-hi), group by 128-wide dst block with group sizes uniform across
    cores, and pack densely (no per-group tile padding).

    Returns (plan, per_core).
    """
    src = np.asarray(edge_index[0], dtype=np.int64)
    dst = np.asarray(edge_index[1], dtype=np.int64)
    w = np.asarray(edge_weight, dtype=np.float64)

    core = dst // NC_N
    drel = dst % NC_N

    deg_full = np.zeros(N, dtype=np.float64)
    np.add.at(deg_full, dst, w)
    dinv_full = 1.0 / np.sqrt(deg_full + 2.0)

    wprime = w * dinv_full[dst]                      # fold dinv[dst] in

    # self-loops are NOT edges here: they are applied as one extra matmul
    # per dst block against the locally-kept node-major t_hat (diag tiles)

    # groups[c][b][s] = (idx int64 rel-to-stream-base, rel f32, w f32)
    groups = [[[None] * 3 for _ in range(NBLK)] for _ in range(P)]
    for c in range(P):
        m = core == c
        s_all = src[m]
        # permuted table row id of each edge's source node
        s_core = s_all // NC_N
        s_rel = s_all % NC_N
        s_row = s_core * NPAD + (s_rel % 128) * NBLK + s_rel // 128
        d_all = drel[m]
        w_all = wprime[m].astype(np.float32)
        blk = d_all // BW
        rel = (d_all % BW).astype(np.float32)
        is_local = (s_all >= c * NC_N) & (s_all < (c + 1) * NC_N)
        for b in range(NBLK):
            mb = blk == b
            mloc = mb & is_local
            mrem = mb & ~is_local
            ridx = np.nonzero(mrem)[0]
            rsrc = s_row[ridx]
            # balance lo/hi: overlap rows [HI_BASE, 32768) go to whichever
            # stream is short on this (core, block)
            fixed_lo = rsrc < HI_BASE
            fixed_hi = rsrc >= 32768
            mid = ~fixed_lo & ~fixed_hi
            n_lo = int(np.clip(len(rsrc) // 2, fixed_lo.sum(),
                               fixed_lo.sum() + mid.sum()))
            take = n_lo - int(fixed_lo.sum())
            mid_idx = np.nonzero(mid)[0]
            to_lo = fixed_lo.copy()
            to_lo[mid_idx[:take]] = True
            for s, ms in (
                (0, np.nonzero(mloc)[0]),
                (1, ridx[to_lo]),
                (2, ridx[~to_lo]),
            ):
                base = c * NPAD if s == 0 else (0 if s == 1 else HI_BASE)
                order = np.argsort(s_row[ms], kind="stable")
                groups[c][b][s] = (
                    (s_row[ms] - base)[order],
                    rel[ms][order],
                    w_all[ms][order],
                )

    # uniform group lengths = max over cores
    n_g = np.zeros((3, NBLK), dtype=np.int64)
    for b in range(NBLK):
        for s in range(3):
            n_g[s, b] = max(len(groups[c][b][s][0]) for c in range(P))

    # group offsets within each packed stream; matmul schedule per block.
    # Boundary tiles shared by two blocks are consumed once per block with
    # full 128 partitions; the other block's edges carry w'=0 in that op's
    # meta column (PE cost depends only on the moving dim, so this is free).
    offs = [0, 0, 0]
    o_g = np.zeros((3, NBLK), dtype=np.int64)
    sched = []          # sched[b] = [(s, tile_col, op_index), ...]
    nop = 0
    for b in range(NBLK):
        ops = []
        for s in range(3):
            o, n = offs[s], int(n_g[s, b])
            o_g[s, b] = o
            offs[s] += n
            if n == 0:
                continue
            for tc_ in range(o // 128, (o + n - 1) // 128 + 1):
                ops.append((s, tc_, nop))
                nop += 1
        sched.append(ops)

    T_s = [int(offs[s]) for s in range(3)]            # packed edge slots
    TS = [max(1, -(-T_s[s] // 128)) for s in range(3)]  # stream tiles

    # gather chunk lists + issue order (lc first, then lo/hi merged by the
    # first block that consumes each chunk)
    def first_block(s, tile0):
        pos = tile0 * 128
        for b in range(NBLK):
            if pos < o_g[s, b] + n_g[s, b]:
                return b
        return NBLK

    chunk_list = []                                   # (s, tile0, ntiles)
    remote = []
    for s in range(3):
        for t0 in range(0, TS[s], C_TILES):
            nt = min(C_TILES, TS[s] - t0)
            if s == 0:
                chunk_list.append((s, t0, nt))
            else:
                remote.append((first_block(s, t0), s, t0, nt))
    remote.sort()
    chunk_list += [(s, t0, nt) for _, s, t0, nt in remote]

    plan = {"n_g": n_g, "o_g": o_g, "sched": sched, "T_s": T_s, "TS": TS,
            "chunks": chunk_list, "nop": nop}

    per_core = []
    for c in range(P):
        idxs = []
        srel = []                      # per-stream per-slot rel / w'
        sww = []
        for s in range(3):
            nbuf = TS[s] * 128
            ib = np.zeros(nbuf, dtype=np.int16)
            mrel = np.zeros(nbuf, dtype=np.float32)
            mw = np.zeros(nbuf, dtype=np.float32)
            for b in range(NBLK):
                idx, rel, ww = groups[c][b][s]
                o, n = int(o_g[s, b]), len(idx)
                ib[o : o + n] = idx.astype(np.int16)
                mrel[o : o + n] = rel
                mw[o : o + n] = ww
            # wrapped int16 index layout: idx i at [i % 16, i // 16],
            # replicated 8x along partitions (one stripe per Q7 core)
            idxs.append(
                np.ascontiguousarray(np.tile(ib.reshape(-1, 16).T, (8, 1)))
            )
            srel.append(mrel)
            sww.append(mw)

        # meta per consumption-order op: [p, 2i] = rel, [p, 2i+1] = w', with
        # w'=0 for slots outside the op's (block, stream) group
        meta = np.zeros((128, 2 * nop), dtype=np.float32)
        for b in range(NBLK):
            for s, tc_, i in sched[b]:
                slots = np.arange(tc_ * 128, tc_ * 128 + 128)
                inside = (slots >= o_g[s, b]) & (slots < o_g[s, b] + n_g[s, b])
                meta[:, 2 * i] = np.where(inside, srel[s][slots], 0.0)
                meta[:, 2 * i + 1] = np.where(inside, sww[s][slots], 0.0)

        dinv_c = np.zeros(NPAD, dtype=np.float32)
        dinv_c[:NC_N] = dinv_full[c * NC_N : (c + 1) * NC_N]
        dinvT = np.ascontiguousarray(dinv_c.reshape(NBLK, 128).T)

        per_core.append(
            {
                "lcidx": idxs[0], "loidx": idxs[1], "hiidx": idxs[2],
                "meta": meta, "dinvT": dinvT,
            }
        )

    return plan, per_core


def _build_program(plan, single_core=False):
    # single_core=True swaps the AllGather for a local DMA copy and builds a
    # 1-device module, so the cost-model TimelineSim (single-core only) can
    # profile the kernel; numerics of remote nodes are wrong in that mode.
    TS = plan["TS"]
    sched = plan["sched"]
    chunk_list = plan["chunks"]
    nc = bacc.Bacc(
        "TRN2",
        target_bir_lowering=False,
        debug=False,
        enable_asserts=False,
        num_devices=1 if single_core else P,
    )

    # ---- I/O -------------------------------------------------------------
    xT_d = nc.dram_tensor("xT", [IN_D, NC_N], BF, kind="ExternalInput")
    idx_d = {
        nm: nc.dram_tensor(nm, [128, TS[s] * 8], I16, kind="ExternalInput")
        for s, nm in enumerate(["lcidx", "loidx", "hiidx"])
    }
    nop = plan["nop"]
    meta_d = nc.dram_tensor("meta", [128, 2 * nop], FP, kind="ExternalInput")
    dinvT_d = nc.dram_tensor("dinvT", [128, NBLK], FP, kind="ExternalInput")
    w_d = {
        name: nc.dram_tensor(name, shape, BF, kind="ExternalInput")
        for name, shape in [
            ("W1", [IN_D, H]),
            ("W2", [H, H]),
            ("W3", [H, H]),
            ("Wg1", [H, H]),
            ("Wg2", [H, H]),
            ("Wg3", [H, H]),
            ("Wg4", [H, H]),
            ("Wh", [H, OUT_D]),
        ]
    }
    # bias columns: 0..2 = b1..b3, 3..6 = bg1..bg4, 7..13 = negated, 14 = bh
    bias_d = nc.dram_tensor("bias", [128, 16], FP, kind="ExternalInput")
    iota_d = nc.dram_tensor("iota128", [128, BW], BF, kind="ExternalInput")
    ident2_d = nc.dram_tensor("ident2", [128, 128], BF, kind="ExternalInput")
    out_d = nc.dram_tensor("out", [OUT_D, NC_N], FP, kind="ExternalOutput")

    rg = [list(range(P))]

    with tile.TileContext(nc) as tc, ExitStack() as ctx:
        const = ctx.enter_context(tc.tile_pool(name="const", bufs=1))
        big = ctx.enter_context(tc.tile_pool(name="big", bufs=1))
        swp = ctx.enter_context(tc.tile_pool(name="swp", bufs=16))
        epp = ctx.enter_context(tc.tile_pool(name="epp", bufs=3))
        ebp = ctx.enter_context(tc.tile_pool(name="ebp", bufs=6))
        vlc_p = ctx.enter_context(tc.tile_pool(name="vlc", bufs=3))
        vlo_p = ctx.enter_context(tc.tile_pool(name="vlo", bufs=3))
        vhi_p = ctx.enter_context(tc.tile_pool(name="vhi", bufs=3))
        ps_dense = ctx.enter_context(tc.tile_pool(name="psd", bufs=3, space="PSUM"))
        ps_blk = ctx.enter_context(tc.tile_pool(name="psb", bufs=3, space="PSUM"))
        ps_tr = ctx.enter_context(tc.tile_pool(name="pst", bufs=2, space="PSUM"))
        dram = ctx.enter_context(tc.tile_pool(name="dram", bufs=2, space="DRAM"))

        # ---- constants (idx/meta are layer-invariant: resident in SBUF) ---
        def load_const(shape, src_ap, name, dtype=FP):
            t = const.tile(shape, dtype, tag=name)
            nc.sync.dma_start(t[:], src_ap)
            return t

        w_sb = {k: load_const(list(v.shape), v[:], k, BF) for k, v in w_d.items()}
        bias = load_const([128, 16], bias_d[:], "bias")
        dinvT = load_const([128, NBLK], dinvT_d[:], "dinvT")
        # gather/scatter-phase constants (idx/meta/iota/ident2/diag) are
        # loaded AFTER the MLP emission so their DMAs fill the MLP window
        # instead of delaying the first x chunks — see load_gather_consts()
        iota = None
        idx_sb = None
        meta_sb = None
        diag_sb = []

        def load_gather_consts():
            nonlocal iota, idx_sb, meta_sb
            iota = load_const([128, BW], iota_d[:], "iota", BF)
            idx_sb = [
                load_const([128, TS[s] * 8], idx_d[nm][:], nm, I16)
                for s, nm in enumerate(["lcidx", "loidx", "hiidx"])
            ]
            meta_sb = load_const([128, 2 * nop], meta_d[:], "meta")
            ident2 = load_const([128, 128], ident2_d[:], "ident2", BF)
            # diag_b = 2*dinv*I, the self-loop scatter tiles (layer-invariant)
            for b in range(NBLK):
                d = const.tile([128, 128], BF, tag=f"diag{b}", name=f"diag{b}")
                nc.vector.tensor_scalar(
                    d[:], ident2[:], dinvT[:, b : b + 1], 0.0,
                    ALU.mult, ALU.add,
                )
                diag_sb.append(d)

        h_sb = big.tile([128, NPAD], BF, tag="h")
        x_sb = big.tile([IN_D, NC_N], BF, tag="x")
        nc.sync.dma_start(x_sb[:], xT_d[:])
        # two node-major t_hat buffers (layer parity): feed the self-loop
        # matmuls and the agin DMA without a DRAM round trip
        t_loc = [
            big.tile([128, NPAD], BF, tag=f"tloc{i}", name=f"tloc{i}")
            for i in range(2)
        ]

        def sw_build(sw4, j, i):
            """Build the one-hot(dst_rel)*w' scatter tile for consumption-
            order op i into slice j of a shared [128, 512] tile (one pool
            ring slot per 4 builds keeps the DVE sequencer clear)."""
            o = 2 * i
            nc.vector.tensor_scalar(
                sw4[:, j * BW : (j + 1) * BW],
                iota[:],
                meta_sb[:, o : o + 1],
                meta_sb[:, o + 1 : o + 2],
                ALU.is_equal,
                ALU.mult,
            )

        # ---- embedding MLP -------------------------------------------------

        def elu_ep(dst_ap, ps_ap, bcol, cw, mode):
            # ELU: out = relu(x+b) + (exp(min(x+b, 0)) - 1), computed from a
            # bf16 biased copy so the DVE legs run in fast (2-byte) mode:
            # ACT does the PSUM read + exp, DVE does min / (max - 1) / add.
            tb = epp.tile([128, 512], BF, tag="tb")
            r1 = epp.tile([128, 512], BF, tag="r")
            nm = epp.tile([128, 512], BF, tag="nm")
            e2 = epp.tile([128, 512], BF, tag="e2")
            if mode == "act":
                nc.scalar.activation(
                    tb[:, :cw], ps_ap, ACT_F.Identity,
                    bias=bias[:, bcol : bcol + 1],
                )
            else:
                # alternate the PSUM read between ACT and DVE so neither
                # engine binds the 39-stage MLP pipeline
                nc.vector.tensor_scalar(
                    tb[:, :cw], ps_ap, bias[:, bcol : bcol + 1], 0.0,
                    ALU.add, ALU.add,
                )
            nc.vector.tensor_scalar(
                nm[:, :cw], tb[:, :cw], 0.0, 0.0, ALU.min, ALU.add
            )
            nc.scalar.activation(e2[:, :cw], nm[:, :cw], ACT_F.Exp)
            nc.vector.tensor_scalar(
                r1[:, :cw], tb[:, :cw], 0.0, -1.0, ALU.max, ALU.add
            )
            nc.vector.tensor_tensor(
                dst_ap, e2[:, :cw], r1[:, :cw], ALU.add
            )

        # ---- t-phase (t_hat table build), interleavable in 4-block groups -
        def emit_t_group(g4, wg, agin, tl, on_dve=False):
            b0 = 4 * g4
            blocks = range(b0, min(b0 + 4, NBLK))
            for nb in blocks:
                trp = ps_tr.tile([128, 128], FP, tag="tr")
                nc.tensor.matmul(
                    trp[:], h_sb[:, nb * 128 : (nb + 1) * 128], wg[:]
                )
                if on_dve:
                    # during the MLP the ACT engine is the bottleneck
                    nc.vector.tensor_scalar(
                        tl[:, nb * 128 : (nb + 1) * 128], trp[:],
                        dinvT[:, nb : nb + 1], 0.0, ALU.mult, ALU.add,
                    )
                else:
                    nc.scalar.activation(
                        tl[:, nb * 128 : (nb + 1) * 128], trp[:], ACT_F.Copy,
                        scale=dinvT[:, nb : nb + 1],
                    )
            cols = len(list(blocks)) * 128
            # agin is the raw partition-major image of t_loc: contiguous copy
            nc.sync.dma_start(
                agin[:, b0 * 128 : b0 * 128 + cols],
                tl[:, b0 * 128 : b0 * 128 + cols],
            )

        def emit_head_chunk(ch):
            off, cw = CHUNKS[ch]
            ps = ps_dense.tile([128, 512], FP, tag="dense")
            nc.tensor.matmul(
                ps[:OUT_D, :cw], w_sb["Wh"][:], h_sb[:, off : off + cw]
            )
            oc = epp.tile([OUT_D, 512], FP, tag="outc")
            nc.scalar.activation(
                oc[:, :cw], ps[:OUT_D, :cw], ACT_F.Identity,
                bias=bias[:OUT_D, 14:15],
            )
            nc.sync.dma_start(out_d[:, off : off + cw], oc[:, :cw])

        # ---- MLP with layer-0 t-phase interleaved -------------------------
        agin = [None] * 4
        tfull = [None] * 4
        agin[0] = dram.tile([128, NPAD], BF, tag="agin", name="agin0")

        for li, (wname, bcol) in enumerate([("W1", 0), ("W2", 1), ("W3", 2)]):
            for ci, (off, cw) in enumerate(CHUNKS):
                if li == 0:
                    rhs = x_sb[:IN_D, off : off + cw]
                    lhs = w_sb["W1"][:IN_D, :]
                else:
                    rhs = h_sb[:, off : off + cw]
                    lhs = w_sb[wname][:]
                ps = ps_dense.tile([128, 512], FP, tag="dense")
                nc.tensor.matmul(ps[:, :cw], lhs, rhs)
                if cw < 512 and li == 2:
                    # zero pad columns of h for the scatter phase
                    nc.vector.memset(h_sb[:, off + cw : NPAD], 0.0)
                elu_ep(
                    h_sb[:, off : off + cw], ps[:, :cw], bcol, cw,
                    mode=("dve", "act")[ci % 2],
                )
                if li == 0 and ci == len(CHUNKS) - 1:
                    load_gather_consts()
                if li == 2 and ci >= 1:
                    emit_t_group(ci - 1, w_sb["Wg1"], agin[0], t_loc[0],
                                 on_dve=True)
        emit_t_group(12, w_sb["Wg1"], agin[0], t_loc[0], on_dve=True)

        # ---- GCN layers ---------------------------------------------------
        for layer in range(4):
            bcol = 3 + layer

            tfull[layer] = dram.tile([P * 128, NPAD], BF, tag="tfull", addr_space="Shared", name=f"tfull{layer}")
            if single_core:
                # dependency-preserving stand-in for the AllGather: the real
                # transfer runs on the collective cores, not the DMA engines,
                # so copy only a token slice that reads every agin write
                # (partitions 0:2 span all column groups) — gathers still
                # wait for the full table build
                nc.sync.dma_start(tfull[layer][:2, :], agin[layer][:2, :])
            else:
                nc.gpsimd.collective_compute(
                    "AllGather",
                    ALU.bypass,
                    replica_groups=rg,
                    ins=[agin[layer][:]],
                    outs=[tfull[layer][:]],
                )

            # gathers in consumption-ish order (lc first: they only need agin)
            tabv = tfull[layer][:].rearrange("p (b f) -> (p b) f", f=128)
            tabs = [
                agin[layer][:].rearrange("p (b f) -> (p b) f", f=128),
                tabv[:32768, :],
                tabv[HI_BASE:, :],
            ]
            pools = [vlc_p, vlo_p, vhi_p]
            vchunks = [{}, {}, {}]          # tile0 -> (tile, nt)
            for s, t0, nt in chunk_list:
                v = pools[s].tile([128, C_TILES, 128], BF, tag=f"v{s}")
                nc.gpsimd.dma_gather(
                    v[:, :nt, :], tabs[s], idx_sb[s][:, t0 * 8 : (t0 + nt) * 8],
                    nt * 128, nt * 128, H, single_packet=False,
                )
                vchunks[s][t0] = v

            if layer < 3:
                agin[layer + 1] = dram.tile([128, NPAD], BF, tag="agin", name=f"agin{layer+1}")
                nwg = w_sb[f"Wg{layer + 2}"]

            # scatter + epilogue (deferred 1 block) + interleaved next t-phase
            aggs = [None] * NBLK

            def emit_epilogue(b):
                agg = aggs[b]
                rb = ebp.tile([128, BW], BF, tag="rb")
                nmb = ebp.tile([128, BW], BF, tag="nmb")
                eb = ebp.tile([128, BW], BF, tag="eb")
                nc.scalar.activation(
                    rb[:], agg[:], ACT_F.Relu, bias=bias[:, bcol : bcol + 1]
                )
                nc.scalar.activation(
                    nmb[:], agg[:], ACT_F.Relu,
                    bias=bias[:, bcol + 7 : bcol + 8], scale=-1.0,
                )
                nc.scalar.activation(eb[:], nmb[:], ACT_F.Exp, scale=-1.0)
                nc.vector.scalar_tensor_tensor(
                    h_sb[:, b * BW : (b + 1) * BW],
                    eb[:], -1.0, rb[:], ALU.add, ALU.add,
                )

            tl_cur = t_loc[layer % 2]
            for b in range(NBLK):
                ops = sched[b]
                agg = ps_blk.tile([128, BW], FP, tag="agg")
                aggs[b] = agg
                nops_b = len(ops)
                # self-loop term: agg = t_loc[b]^T @ (2*dinv*I)
                nc.tensor.matmul(
                    agg[:], tl_cur[:, b * 128 : (b + 1) * 128], diag_sb[b][:],
                    start=True, stop=(nops_b == 0),
                )
                sw4 = None
                for k, (s, tc_, i) in enumerate(ops):
                    j = k % 4
                    if j == 0:
                        sw4 = swp.tile([128, 4 * BW], BF, tag="sw")
                    sw_build(sw4, j, i)
                    t0 = (tc_ // C_TILES) * C_TILES
                    v = vchunks[s][t0]
                    nc.tensor.matmul(
                        agg[:], v[:, tc_ - t0, :],
                        sw4[:, j * BW : (j + 1) * BW],
                        start=False, stop=(k == nops_b - 1),
                    )
                if b >= 1:
                    emit_epilogue(b - 1)
                # interleaved t-phase of the next layer (or head chunks)
                if b >= TLAG and (b - TLAG + 1) % 4 == 0:
                    g4 = (b - TLAG + 1) // 4 - 1
                    if layer < 3:
                        emit_t_group(g4, nwg, agin[layer + 1],
                                     t_loc[(layer + 1) % 2])
                    else:
                        emit_head_chunk(g4)
            emit_epilogue(NBLK - 1)
            # trailing t-groups (blocks whose h finished at the very end)
            gfirst = (NBLK - TLAG) // 4  # first group not yet emitted
            for g4 in range(gfirst, 13):
                if layer < 3:
                    emit_t_group(g4, nwg, agin[layer + 1],
                                 t_loc[(layer + 1) % 2])
                else:
                    emit_head_chunk(g4)

    nc.compile()
    return nc


def _make_in_maps(inputs, per_core):
    import ml_dtypes

    x = np.asarray(inputs["x"], dtype=np.float32)
    bias = np.zeros((128, 16), dtype=np.float32)
    for j, nm in enumerate(["b1", "b2", "b3", "bg1", "bg2", "bg3", "bg4"]):
        b = np.asarray(inputs[nm], dtype=np.float32)
        bias[:, j] = b
        bias[:, j + 7] = -b
    bias[:OUT_D, 14] = np.asarray(inputs["bh"], dtype=np.float32)

    shared = {
        "bias": bias,
        "iota128": np.tile(
            np.arange(BW, dtype=np.float32), (128, 1)
        ).astype(ml_dtypes.bfloat16),
        "ident2": (2.0 * np.eye(128, dtype=np.float32)).astype(
            ml_dtypes.bfloat16
        ),
    }
    for nm in ["W1", "W2", "W3", "Wg1", "Wg2", "Wg3", "Wg4", "Wh"]:
        shared[nm] = np.ascontiguousarray(
            np.asarray(inputs[nm], np.float32)
        ).astype(ml_dtypes.bfloat16)

    in_maps = []
    for c in range(P):
        m = dict(shared)
        m["xT"] = np.ascontiguousarray(
            x[c * NC_N : (c + 1) * NC_N].T
        ).astype(ml_dtypes.bfloat16)
        m.update(per_core[c])
        in_maps.append(m)
    return in_maps


def run(inputs, trace=False):
    """Run the distributed kernel; returns (out [N, OUT_D] fp32, results)."""
    plan, per_core = _prep_edges(inputs["edge_index"], inputs["edge_weight"])
    nc = _build_program(plan)
    in_maps = _make_in_maps(inputs, per_core)
    res = run_bass_kernel_spmd(nc, in_maps, list(range(P)), trace=trace)
    out = np.concatenate(
        [res.results[c]["out"].T for c in range(P)], axis=0
    ).astype(np.float32)
    return out, res


def kernel(**inputs):
    out, _ = run(inputs, trace=False)
    return out
